# revision 23
# baseline (speedup 1.0000x reference)
"""AtomicNumberPooling Trainium2 kernel (v12 — DVE one-hot + SWDGE assist).

Math (from the reference):
    keys   = batch * 100 + (z - 1)                    # per-node (graph, bin) id
    sums   = segment_sum(out, keys, G * 100)          # [G*100, D]
    counts = nodes per graph                          # [G]
    pooled = sums.reshape(G, 100 * D) / max(counts, 1)

Strategy: data-parallel over graphs — 64 graphs per NeuronCore, one
128-row chunk per graph (x pre-scaled by 1/count on host; >128-node
graphs get overflow chunks merged back on the host).  Per chunk:
onehot[128,100].T @ x[128,64] -> the graph's [100,64] block in PSUM,
drained to bf16 and streamed out.

v12 keeps v6's on-device one-hot architecture — HW measurement showed
this box's DMA queues sustain only ~120-200GB/s each with 2.5-4us
completion-semaphore lag, so shipping host-built one-hots (v8-v11)
always loses to building them from the tiny z columns on chip — and
fixes v6's two real flaws:

 1. one-hot supply was DVE-only at ~114ns/chunk vs the 87ns/chunk
    matmul pitch, so the stream stalled ~1.6us.  Now the host packs the
    one-hot for the last 4 chunks of each 16-chunk PSUM group as uint8
    (16 chunks, 205KB) and two small SWDGE cast-DMAs deliver them over
    the otherwise-idle third DMA queue; DVE builds only ~49 chunks and
    stays ahead of the stream.  Late x pieces also ride SWDGE so the
    two HWDGE rings (~230GB/s combined) don't gate the stream tail.
 2. the post-stream tail was a 1024-col drain + 204KB store + receipt
    (~4.9us).  Now PSUM groups run in natural order with the tiny
    overflow group last (drain ~250ns + small store), the last big
    group's drain splits at the PSUM bank boundary between ScalarE and
    DVE, its store splits across both HWDGE rings, and mid-stream
    stores ride SWDGE so they never queue ahead of the final ones.
"""

import bisect
import dataclasses

import numpy as np

NUM_Z = 100
G = 512
P = 128
NCORES = 8
GL = G // NCORES  # graphs per core
PB = 16           # chunks per PSUM tile (2 banks)

# filled by kernel() for optional inspection by a test harness
LAST_RESULTS = None


def _view3(ap, n, w):
    """[128, n*w] access pattern -> [128, n, w]."""
    return dataclasses.replace(ap, ap=[ap.ap[0], (w, n), (1, w)])


def _bcast_inner(ap, w):
    """[128, n] access pattern -> [128, n, w] with stride-0 inner dim."""
    return dataclasses.replace(ap, ap=list(ap.ap) + [(0, w)])


def _bcast_mid(ap, n):
    """[128, w] access pattern -> [128, n, w] with stride-0 middle dim."""
    return dataclasses.replace(ap, ap=[ap.ap[0], (0, n), ap.ap[1]])


def _x_plan(C):
    """x DMA pieces: [1, 3, 12, 16, 16, rest].  First pieces small so the
    first matmul fires early.  The first four ride the HWDGE rings, the
    rest ride SWDGE (third queue)."""
    pat = [1, 3, 12, 16, 16, 12]
    out = []
    c = 0
    for n in pat:
        if c >= C:
            break
        n = min(n, C - c)
        out.append((c, n))
        c += n
    if c < C:
        out.append((c, C - c))
    return out


def _sw_chunks(C):
    """Chunks whose one-hot comes from the host via SWDGE cast-DMA: the
    last 4 of each full 16-chunk group (PE reaches them latest, and the
    tiny transfers land well before that)."""
    return [c for c in range(min(C, 64)) if c % 16 >= 12]


def _build_program(C, D):
    import concourse.bacc as bacc
    import concourse.mybir as mybir
    import concourse.tile as tile

    f32 = mybir.dt.float32
    bf16 = mybir.dt.bfloat16
    u8 = mybir.dt.uint8
    nc = bacc.Bacc("TRN2", debug=False, num_devices=NCORES)

    xps = _x_plan(C)
    sw = _sw_chunks(C)
    sw_pos = {c: i for i, c in enumerate(sw)}
    # dram layout: [zb for ALL chunks | per-piece x blocks].  All z
    # columns ride the first (tiny) piece so DVE can build every one-hot
    # as soon as piece 0 lands — the late pieces' one-hots no longer
    # trail the late x semaphores.
    x_d = nc.dram_tensor("x", [P, C * D + C], bf16, kind="ExternalInput")
    oh_d = nc.dram_tensor(
        "oh", [P, max(1, len(sw)) * NUM_Z], u8, kind="ExternalInput"
    )
    y_d = nc.dram_tensor("y", [NUM_Z, C * D], bf16, kind="ExternalOutput")

    NGRP = (C + PB - 1) // PB

    with tile.TileContext(nc) as tc:
        with (
            tc.tile_pool(name="const", bufs=1) as constp,
            tc.tile_pool(name="xin", bufs=1) as xp,
            tc.tile_pool(name="oh", bufs=1) as ohp,
            tc.tile_pool(name="stage", bufs=NGRP) as stp,
            tc.tile_pool(name="psum", bufs=4, space="PSUM") as pp,
        ):
            # small on-chip iota 0..99 (exact in bf16)
            iota_t = constp.tile([P, NUM_Z], bf16)
            nc.gpsimd.iota(
                iota_t[:], pattern=[[1, NUM_Z]], base=0,
                channel_multiplier=0, allow_small_or_imprecise_dtypes=True,
            )

            # x pieces 0-3 and the small tail piece ride the HWDGE
            # rings; mid pieces ride SWDGE.  On the GpSimd FIFO the tiny
            # one-hot pieces are issued BEFORE the big mid x pieces
            # (needed from chunk 12 vs chunk 32).  dram layout per
            # piece: [zb cols | x cols].
            offs = []
            off = 0
            for i, (c0, cn) in enumerate(xps):
                offs.append(off)
                off += cn * D + (C if i == 0 else 0)
            hw = [i for i in range(len(xps)) if i < 4 or i >= len(xps) - 2]
            xts = [None] * len(xps)
            for r, i in enumerate(hw):
                c0, cn = xps[i]
                w = cn * D + (C if i == 0 else 0)
                xt = xp.tile([P, w], bf16, name=f"x{i}")
                eng = nc.sync if r % 2 == 0 else nc.scalar
                eng.dma_start(xt[:], x_d[:, offs[i] : offs[i] + w])
                xts[i] = xt

            # SWDGE one-hot pieces: two cast-DMAs of ~8 chunks each
            oh_tiles = {}  # chunk -> (tile, col offset)
            if sw:
                for k, (s0, s1) in enumerate([(0, len(sw))]):
                    if s1 <= s0:
                        continue
                    t = ohp.tile(
                        [P, (s1 - s0) * NUM_Z], bf16, name=f"sw{k}"
                    )
                    nc.gpsimd.dma_start(
                        t[:], oh_d[:, s0 * NUM_Z : s1 * NUM_Z]
                    )
                    for kk in range(s1 - s0):
                        oh_tiles[sw[s0 + kk]] = (t, kk)

            # mid x pieces on SWDGE, after the one-hot pieces
            for i, (c0, cn) in enumerate(xps):
                if xts[i] is None:
                    xt = xp.tile([P, cn * D], bf16, name=f"x{i}")
                    nc.gpsimd.dma_start(
                        xt[:], x_d[:, offs[i] : offs[i] + cn * D]
                    )
                    xts[i] = xt

            # DVE-built one-hots: per x-piece, the non-SWDGE chunks, in
            # <=4-chunk is_equal batches so supply tracks consumption
            x_starts = [s for s, _ in xps]
            for i, (c0, cn) in enumerate(xps):
                dve = [c for c in range(c0, c0 + cn) if c not in sw_pos]
                if not dve:
                    continue
                oh = ohp.tile([P, len(dve) * NUM_Z], bf16, name=f"oh{i}")
                runs = [[dve[0]]]
                for c in dve[1:]:
                    if c == runs[-1][-1] + 1:
                        runs[-1].append(c)
                    else:
                        runs.append([c])
                pos = 0
                for run in runs:
                    for s in range(0, len(run), 4):
                        sub = run[s : s + 4]
                        n = len(sub)
                        zcol = sub[0]
                        nc.vector.tensor_tensor(
                            _view3(
                                oh[:, pos * NUM_Z : (pos + n) * NUM_Z],
                                n, NUM_Z,
                            ),
                            _bcast_inner(
                                xts[0][:, zcol : zcol + n], NUM_Z
                            ),
                            _bcast_mid(iota_t[:, :], n),
                            mybir.AluOpType.is_equal,
                        )
                        for kk, c in enumerate(sub):
                            oh_tiles[c] = (oh, pos + kk)
                        pos += n

            # matmul stream + per-psum-group drain/store, natural order
            for g in range(NGRP):
                c0 = g * PB
                cn = min(PB, C - c0)
                ps = pp.tile([P, cn * D], f32)
                for jj in range(cn):
                    j = c0 + jj
                    oht, toff = oh_tiles[j]
                    xi = bisect.bisect_right(x_starts, j) - 1
                    xs0, xn = xps[xi]
                    zoff = C if xi == 0 else 0
                    nc.tensor.matmul(
                        out=ps[:NUM_Z, jj * D : (jj + 1) * D],
                        lhsT=oht[:, toff * NUM_Z : (toff + 1) * NUM_Z],
                        rhs=xts[xi][
                            :,
                            zoff + (j - xs0) * D : zoff + (j - xs0 + 1) * D,
                        ],
                        start=True,
                        stop=True,
                    )
                cols = cn * D
                late = g >= NGRP - 2
                if cols > 512 and late:
                    # last big group: drain halves at the PSUM bank
                    # boundary into SEPARATE stage tiles (one tile would
                    # add a false whole-tile WAW dep serializing the
                    # halves), each stored immediately on its own ring
                    h = 512
                    stA = stp.tile([P, h], bf16, name=f"stA{g}")
                    stB = stp.tile([P, cols - h], bf16, name=f"stB{g}")
                    nc.scalar.copy(stA[:NUM_Z, :], ps[:NUM_Z, :h])
                    nc.sync.dma_start(
                        y_d[:, c0 * D : c0 * D + h], stA[:NUM_Z, :]
                    )
                    nc.vector.tensor_copy(
                        stB[:NUM_Z, :], ps[:NUM_Z, h:cols]
                    )
                    nc.scalar.dma_start(
                        y_d[:, c0 * D + h : c0 * D + cols], stB[:NUM_Z, :]
                    )
                else:
                    stage = stp.tile([P, cn * D], bf16, name=f"st{g}")
                    nc.scalar.copy(
                        stage[:NUM_Z, :cols], ps[:NUM_Z, :cols]
                    )
                    if late:
                        nc.sync.dma_start(
                            y_d[:, c0 * D : c0 * D + cols],
                            stage[:NUM_Z, :cols],
                        )
                    else:
                        # mid-stream stores ride SWDGE from GpSimd
                        nc.gpsimd.dma_start(
                            y_d[:, c0 * D : c0 * D + cols],
                            stage[:NUM_Z, :cols],
                        )
    nc.compile()
    return nc


def _prep(x, z, b, D):
    """Build per-core padded inputs.  Returns (in_maps, over_maps, C)."""
    import ml_dtypes

    counts = np.bincount(b, minlength=G).astype(np.int64)
    starts = np.zeros(G + 1, np.int64)
    np.cumsum(counts, out=starts[1:])
    inv = 1.0 / np.maximum(counts, 1).astype(np.float32)
    xs = (x * inv[b][:, None]).astype(ml_dtypes.bfloat16)

    per_core = []
    for k in range(NCORES):
        main = []  # (node_start, length, graph) — one per graph, in order
        over = []  # extra pieces for graphs with >P nodes
        for gl in range(GL):
            g = k * GL + gl
            s, n = int(starts[g]), int(counts[g])
            main.append((s, min(n, P), g))
            off = P
            while off < n:
                over.append((s + off, min(n - off, P), g))
                off += P
        per_core.append((main, over))

    B = max(len(o) for _, o in per_core)
    C = GL + B

    xps = _x_plan(C)
    sw = _sw_chunks(C)
    in_maps, over_maps = [], []
    for k in range(NCORES):
        main, over = per_core[k]
        chunks = main + over
        xT = np.zeros((P, C, D), ml_dtypes.bfloat16)
        zb = np.full((P, C), -1.0, ml_dtypes.bfloat16)
        zi = np.full((P, C), -1, np.int64)
        for j, (s, ln, g) in enumerate(chunks):
            xT[:ln, j, :] = xs[s : s + ln]
            zb[:ln, j] = z[s : s + ln]
            zi[:ln, j] = z[s : s + ln]
        parts = [zb]
        for c0, cn in xps:
            parts.append(xT[:, c0 : c0 + cn, :].reshape(P, cn * D))
        oh = np.zeros((P, max(1, len(sw)), NUM_Z), np.uint8)
        for i, c in enumerate(sw):
            rr = np.nonzero(zi[:, c] >= 0)[0]
            oh[rr, i, zi[rr, c]] = 1
        in_maps.append(
            {
                "x": np.ascontiguousarray(np.concatenate(parts, axis=1)),
                "oh": np.ascontiguousarray(
                    oh.reshape(P, max(1, len(sw)) * NUM_Z)
                ),
            }
        )
        over_maps.append([(GL + j, g) for j, (s, ln, g) in enumerate(over)])
    return in_maps, over_maps, C


def _ensure_ntff_hook():
    """run_bass_kernel_spmd(trace=True) under axon imports antenv.axon_hooks,
    which this agent image lacks — recreate it (with the ctypes NTFF hook if
    available) so a BASS_TRACE=1 environment doesn't crash kernel()."""
    import sys
    import types

    try:
        import antenv.axon_hooks  # noqa: F401

        return
    except ImportError:
        pass
    try:
        import antenv
    except ImportError:
        return
    hook = None
    try:
        from trn_agent_boot.trn_boot import _ntff_profile_via_ctypes

        hook = _ntff_profile_via_ctypes("/opt/axon/libaxon_pjrt.so")
    except Exception:
        pass
    mod = types.ModuleType("antenv.axon_hooks")
    mod._hook = hook
    mod.get_axon_ntff_profile_hook = lambda: mod._hook
    mod.set_axon_ntff_profile_hook = lambda h: setattr(mod, "_hook", h)
    sys.modules["antenv.axon_hooks"] = mod
    antenv.axon_hooks = mod


def kernel(out, z_rv, x_rv_batch):
    global LAST_RESULTS
    from concourse.bass_utils import run_bass_kernel_spmd

    _ensure_ntff_hook()

    x = np.ascontiguousarray(np.asarray(out), dtype=np.float32)
    z = np.asarray(z_rv).astype(np.int64) - 1  # 0..99
    b = np.asarray(x_rv_batch).astype(np.int64)
    D = x.shape[1]

    in_maps, over_maps, C = _prep(x, z, b, D)
    nc = _build_program(C, D)
    res = run_bass_kernel_spmd(nc, in_maps, core_ids=list(range(NCORES)))
    LAST_RESULTS = res

    full = np.empty((G, NUM_Z * D), np.float32)
    for k in range(NCORES):
        yk = np.asarray(res.results[k]["y"]).astype(np.float32)
        blocks = (
            yk.reshape(NUM_Z, C, D).transpose(1, 0, 2).reshape(C, NUM_Z * D)
        )
        full[k * GL : (k + 1) * GL] = blocks[:GL]
        for j, g in over_maps[k]:
            full[g] += blocks[j]
    return full
